# revision 1
# baseline (speedup 1.0000x reference)
"""Causal self-attention (RMS-normed QK + partial RoPE + lambda-blended V)
for Trainium2, tensor-parallel over heads across 8 NeuronCores.

Per core: 2 heads. Device pipeline per 512-token block:
  x -> bf16 -> DRAM scratch -> DMA-transpose -> xT
  QKV matmuls (bf16, fp32 psum) -> RMS (ACT square+accum, exp(-.5 ln)) -> RoPE (DVE)
  -> PE-transpose q,k -> flash-style causal attention (scores^T, ACT exp,
  mask, fp32 l accumulation, PV accumulation in psum) -> per-head 1/l fold
  into output projection -> partial out [T, D] fp32.
Host: shards weights per core, sums the 8 partial outputs.
"""
import sys
sys.path.insert(0, "/opt/trn_rl_repo")

import math
import numpy as np
import ml_dtypes

import concourse.bass as bass
import concourse.tile as tile
from concourse import bacc, mybir
from concourse.masks import make_identity

bf16 = ml_dtypes.bfloat16
F32 = mybir.dt.float32
BF = mybir.dt.bfloat16
AF = mybir.ActivationFunctionType
ALU = mybir.AluOpType

D = 2048          # model dim
NH = 16           # total heads
DH = 128          # head dim
NCORES = 8
HPC = NH // NCORES          # heads per core = 2
DLOC = HPC * DH             # local hdim = 256
EPS = 1e-6
TB = 512                    # t-block size
SQRT_DH = math.sqrt(DH)

_BUILD_CACHE = {}


def _build(T):
    """Build the per-core Bass program (same program on all cores)."""
    NTB = T // TB
    nc = bacc.Bacc("TRN2", target_bir_lowering=False)

    x_in = nc.dram_tensor("x", [T, D], F32, kind="ExternalInput")
    wq_in = nc.dram_tensor("wqkv", [D, 3 * DLOC], BF, kind="ExternalInput")
    wp_in = nc.dram_tensor("wproj", [DLOC, D], BF, kind="ExternalInput")
    ve_in = nc.dram_tensor("ve", [T, DLOC], BF, kind="ExternalInput")
    cos_in = nc.dram_tensor("cos", [T, 32], F32, kind="ExternalInput")
    sin_in = nc.dram_tensor("sin", [T, 32], F32, kind="ExternalInput")
    mask_in = nc.dram_tensor("mask", [128, 4, TB], BF, kind="ExternalInput")
    out_d = nc.dram_tensor("out", [T, D], F32, kind="ExternalOutput")

    with tile.TileContext(nc) as tc:
        with (
            tc.tile_pool(name="const", bufs=1) as const,
            tc.tile_pool(name="res", bufs=1) as res,
            tc.tile_pool(name="xa", bufs=2) as xa,
            tc.tile_pool(name="xb", bufs=2) as xb,
            tc.tile_pool(name="xt", bufs=1) as xtp,
            tc.tile_pool(name="work", bufs=2) as work,
            tc.tile_pool(name="att", bufs=3) as att,
            tc.tile_pool(name="accp", bufs=2) as accp,
            tc.tile_pool(name="prj", bufs=2) as prj,
            tc.tile_pool(name="psA", bufs=2, space="PSUM") as psA,
            tc.tile_pool(name="psB", bufs=1, space="PSUM") as psB,
            tc.tile_pool(name="psC", bufs=1, space="PSUM") as psC,
            tc.tile_pool(name="psD", bufs=2, space="PSUM") as psD,
            tc.tile_pool(name="dram", bufs=1, space="DRAM") as dram,
        ):
            # ---------------- constants ----------------
            wq_sb = const.tile([128, D // 128, 3 * DLOC], BF, tag="wq")
            nc.sync.dma_start(wq_sb[:], wq_in.rearrange("(c p) e -> p c e", p=128))
            wp_sb = const.tile([128, HPC, D], BF, tag="wp")
            nc.sync.dma_start(wp_sb[:], wp_in.rearrange("(h p) e -> p h e", p=128))
            cos_sb = const.tile([128, T // 128, 32], F32, tag="cos")
            nc.sync.dma_start(cos_sb[:], cos_in.rearrange("(c p) f -> p c f", p=128))
            sin_sb = const.tile([128, T // 128, 32], F32, tag="sin")
            nc.sync.dma_start(sin_sb[:], sin_in.rearrange("(c p) f -> p c f", p=128))
            mask_sb = const.tile([128, 4, TB], BF, tag="mask")
            nc.sync.dma_start(mask_sb[:], mask_in[:])
            ident = const.tile([128, 128], BF, tag="ident")
            make_identity(nc, ident[:])
            ones = const.tile([128, 1], F32, tag="ones")
            nc.vector.memset(ones[:], 1.0)
            lnbias = const.tile([128, 1], F32, tag="lnbias")
            nc.vector.memset(lnbias[:], float(EPS * SQRT_DH))

            # ---------------- resident per-block tensors ----------------
            qT = [res.tile([128, HPC, TB], BF, tag=f"qT{i}", name=f"qT{i}") for i in range(NTB)]
            kT = [res.tile([128, HPC, TB], BF, tag=f"kT{i}", name=f"kT{i}") for i in range(NTB)]
            vB = [res.tile([128, 4, DLOC], BF, tag=f"v{i}", name=f"v{i}") for i in range(NTB)]
            xdr = [dram.tile([TB, D], BF, tag=f"xdr{i}", name=f"xdr{i}") for i in range(NTB)]

            for ti in range(NTB):
                t0 = ti * TB
                # ============ QKV stage ============
                for sub in range(4):
                    x_nat = xa.tile([128, D], F32, tag="xnat")
                    nc.sync.dma_start(x_nat[:], x_in[t0 + sub * 128: t0 + (sub + 1) * 128, :])
                    x_bfs = xb.tile([128, D], BF, tag="xbf")
                    nc.gpsimd.tensor_copy(x_bfs[:], x_nat[:])
                    nc.sync.dma_start(xdr[ti][sub * 128:(sub + 1) * 128, :], x_bfs[:])
                xt = xtp.tile([128, D // 128, TB], BF, tag="xt")
                for dc in range(D // 128):
                    nc.sync.dma_start_transpose(xt[:, dc, :], xdr[ti][:, dc * 128:(dc + 1) * 128])

                for sub in range(4):
                    tg = ti * 4 + sub
                    qkv_ps = psA.tile([128, 1024], F32, tag="big")
                    ndc = D // 128
                    for dc in range(ndc):
                        lhsT = xt[:, dc, sub * 128:(sub + 1) * 128]
                        st, sp = dc == 0, dc == ndc - 1
                        # q|k share psum bank 0 as one N=512 group; v is bank 1
                        nc.tensor.matmul(qkv_ps[:, 0:512], lhsT, wq_sb[:, dc, 0:512], start=st, stop=sp)
                        nc.tensor.matmul(qkv_ps[:, 512:768], lhsT, wq_sb[:, dc, 512:768], start=st, stop=sp)
                    # v (lambda0 pre-folded in weights; lambda1*ve DMA-accumulated below)
                    nc.any.tensor_copy(vB[ti][:, sub, :], qkv_ps[:, 512:768])
                    # rms statistics: sumsq per head for q and k
                    ssq = work.tile([128, 4], F32, tag="ssq")
                    sq_scr = work.tile([128, 512], BF, tag="sqscr")
                    for i in range(4):
                        nc.scalar.activation(
                            sq_scr[:, i * 128:(i + 1) * 128], qkv_ps[:, i * 128:(i + 1) * 128],
                            AF.Square, accum_out=ssq[:, i:i + 1])
                    # rstd' = (sqrt(DH)*(ms+eps))^-1/2 = exp(-0.5*ln(...)); folds the
                    # 1/sqrt(DH) score scale (split as DH^-0.25 into q and k each)
                    rstd = work.tile([128, 4], F32, tag="rstd")
                    nc.scalar.activation(rstd[:], ssq[:], AF.Ln,
                                         scale=float(SQRT_DH / DH), bias=lnbias[:])
                    nc.scalar.activation(rstd[:], rstd[:], AF.Exp, scale=-0.5)
                    # normalize + cast
                    qn = work.tile([128, HPC, DH], BF, tag="qn")
                    kn = work.tile([128, HPC, DH], BF, tag="kn")
                    for h in range(HPC):
                        nc.any.tensor_scalar_mul(qn[:, h, :], qkv_ps[:, h * 128:(h + 1) * 128], rstd[:, h:h + 1])
                        nc.any.tensor_scalar_mul(kn[:, h, :], qkv_ps[:, 256 + h * 128:256 + (h + 1) * 128], rstd[:, 2 + h:3 + h])
                    # rope (first 32 freq pairs only; rest are identity)
                    cosb = cos_sb[:, tg, :][:, None, :].broadcast_to([128, HPC, 32])
                    sinb = sin_sb[:, tg, :][:, None, :].broadcast_to([128, HPC, 32])
                    for tl in (qn, kn):
                        x1 = tl[:, :, 0:32]
                        x2 = tl[:, :, 64:96]
                        r1 = work.tile([128, HPC, 32], BF, tag="r1")
                        r2 = work.tile([128, HPC, 32], BF, tag="r2")
                        r3 = work.tile([128, HPC, 32], BF, tag="r3")
                        r4 = work.tile([128, HPC, 32], BF, tag="r4")
                        nc.vector.tensor_mul(r1[:], x1, cosb)
                        nc.vector.tensor_mul(r2[:], x2, sinb)
                        nc.vector.tensor_mul(r3[:], x1, sinb)
                        nc.vector.tensor_mul(r4[:], x2, cosb)
                        nc.vector.tensor_add(x1, r1[:], r2[:])
                        nc.vector.tensor_sub(x2, r4[:], r3[:])
                    # transpose q,k into resident [d, t] layout
                    for h in range(HPC):
                        for tl, dstl in ((qn, qT), (kn, kT)):
                            tp = psC.tile([128, 128], BF, tag="tp")
                            nc.tensor.transpose(tp[:], tl[:, h, :], ident[:])
                            nc.any.tensor_copy(dstl[ti][:, h, sub * 128:(sub + 1) * 128], tp[:])
                # blend ve into v via accumulating DMA
                nc.gpsimd.dma_start(
                    vB[ti][:], ve_in[t0:t0 + TB, :].rearrange("(c p) d -> p c d", p=128),
                    accum_op=ALU.add)

                # ============ attention stage ============
                linv = prj.tile([128, HPC, 4], F32, tag="linv")
                oB = prj.tile([128, HPC, TB], BF, tag="o")
                ns = (ti + 1) * 4
                for h in range(HPC):
                    l_acc = accp.tile([128, TB], F32, tag="lacc")
                    nc.vector.memset(l_acc[:], 0.0)
                    o_ps = psB.tile([128, TB], F32, tag="o")
                    for sj2 in range(0, ns, 2):
                        sc_ps = psA.tile([128, 1024], F32, tag="big")
                        for k2 in range(2):
                            sj = sj2 + k2
                            blk, sb_ = sj // 4, sj % 4
                            nc.tensor.matmul(
                                sc_ps[:, k2 * 512:(k2 + 1) * 512],
                                kT[blk][:, h, sb_ * 128:(sb_ + 1) * 128],
                                qT[ti][:, h, :], start=True, stop=True)
                        probs = att.tile([128, 1024], BF, tag="probs")
                        nc.scalar.activation(probs[:], sc_ps[:], AF.Exp)
                        for k2 in range(2):
                            j = sj2 + k2 - ti * 4
                            if j >= 0:  # diagonal block: causal mask
                                nc.vector.tensor_mul(
                                    probs[:, k2 * 512:(k2 + 1) * 512],
                                    probs[:, k2 * 512:(k2 + 1) * 512], mask_sb[:, j, :])
                        nc.vector.tensor_add(l_acc[:], l_acc[:], probs[:, 0:512])
                        nc.vector.tensor_add(l_acc[:], l_acc[:], probs[:, 512:1024])
                        for k2 in range(2):
                            sj = sj2 + k2
                            blk, sb_ = sj // 4, sj % 4
                            nc.tensor.matmul(
                                o_ps[:], vB[blk][:, sb_, h * 128:(h + 1) * 128],
                                probs[:, k2 * 512:(k2 + 1) * 512],
                                start=(sj == 0), stop=(sj == ns - 1))
                    # transposed partition-reduce of l (fp32 matmul, N=1)
                    lcol = psC.tile([128, 4], F32, tag="tp")
                    for c in range(4):
                        nc.tensor.matmul(lcol[:, c:c + 1], l_acc[:, c * 128:(c + 1) * 128],
                                         ones[:], start=(c == 0), stop=(c == 3))
                    nc.vector.reciprocal(linv[:, h, :], lcol[:])
                    nc.any.tensor_copy(oB[:, h, :], o_ps[:])

                # ============ projection stage ============
                for sub in range(4):
                    out_sb = prj.tile([128, D], F32, tag="outsb")
                    for dn in range(D // 512):
                        pr0 = psD.tile([128, 512], F32, tag="pr")
                        nc.tensor.matmul(pr0[:], oB[:, 0, sub * 128:(sub + 1) * 128],
                                         wp_sb[:, 0, dn * 512:(dn + 1) * 512], start=True, stop=True)
                        tmp = prj.tile([128, 512], F32, tag="tmp")
                        nc.any.tensor_scalar_mul(tmp[:], pr0[:], linv[:, 0, sub:sub + 1])
                        pr1 = psD.tile([128, 512], F32, tag="pr")
                        nc.tensor.matmul(pr1[:], oB[:, 1, sub * 128:(sub + 1) * 128],
                                         wp_sb[:, 1, dn * 512:(dn + 1) * 512], start=True, stop=True)
                        nc.vector.scalar_tensor_tensor(
                            out_sb[:, dn * 512:(dn + 1) * 512], pr1[:], linv[:, 1, sub:sub + 1],
                            tmp[:], op0=ALU.mult, op1=ALU.add)
                    nc.sync.dma_start(out_d[t0 + sub * 128: t0 + (sub + 1) * 128, :], out_sb[:])
    return nc


def _host_prep(x, ve, lambdas, qkv_w, proj_w, T):
    """Build the 8 per-core input maps (sharding + constant tables)."""
    x = np.ascontiguousarray(np.asarray(x, np.float32).reshape(T, D))
    ve = np.asarray(ve, np.float32).reshape(T, NH * DH)
    lam = np.asarray(lambdas, np.float32)
    qkv_w = np.asarray(qkv_w, np.float32)
    proj_w = np.asarray(proj_w, np.float32)

    quarter = DH // 4
    ang = (1.0 / 1024.0) ** np.linspace(0.0, 1.0, quarter, dtype=np.float32)
    theta = np.arange(T, dtype=np.float32)[:, None] * ang[None, :]   # [T, 32]
    cos_t = np.cos(theta).astype(np.float32)
    sin_t = np.sin(theta).astype(np.float32)

    s_l = np.arange(128)[:, None]
    t_l = np.arange(TB)[None, :]
    mask = np.stack([(t_l >= s_l + 128 * j) for j in range(4)], axis=1).astype(bf16)  # [128,4,TB]

    in_maps = []
    for c in range(NCORES):
        sl = slice(c * DLOC, (c + 1) * DLOC)
        wqkv = np.concatenate(
            [qkv_w[0, sl].T, qkv_w[1, sl].T, lam[0] * qkv_w[2, sl].T], axis=1)  # [D, 768]
        in_maps.append({
            "x": x,
            "wqkv": np.ascontiguousarray(wqkv).astype(bf16),
            "wproj": np.ascontiguousarray(proj_w[:, sl].T).astype(bf16),
            "ve": np.ascontiguousarray(lam[1] * ve[:, sl]).astype(bf16),
            "cos": cos_t, "sin": sin_t, "mask": mask,
        })
    return in_maps


def kernel(x, ve, lambdas, qkv_w, proj_w):
    B, T, _ = x.shape
    in_maps = _host_prep(x, ve, lambdas, qkv_w, proj_w, T)
    if T not in _BUILD_CACHE:
        nc = _build(T)
        nc.compile()
        _BUILD_CACHE[T] = nc
    nc = _BUILD_CACHE[T]

    from concourse.bass_utils import run_bass_kernel_spmd
    res = run_bass_kernel_spmd(nc, in_maps, core_ids=list(range(NCORES)))
    out = np.zeros((T, D), np.float32)
    for c in range(NCORES):
        out += res.results[c]["out"]
    return out.reshape(B, T, D)



# revision 5
# speedup vs baseline: 1.3930x; 1.3930x over previous
"""Causal self-attention (RMS-normed QK + partial RoPE + lambda-blended V)
for Trainium2, tensor-parallel over heads across 8 NeuronCores.

Per core: 2 heads. Host pre-transposes x to xT (bf16), so the device
pipeline per 512-token block is:
  xT strip DMA -> QKV matmuls (bf16, fp32 psum) -> RMS via ACT square-accum
  + DVE Newton rsqrt (no activation-table thrash) -> RoPE (DVE, fused q|k)
  -> PE-transpose q,k -> flash-style causal attention with diagonal-block
  trimming (scores^T, ACT exp, triangular mask, fp32 l accumulation on DVE)
  -> l row-reduce + 1/l broadcast built on PE -> pre-scaled o -> fused
  two-head output projection -> partial out [T, D] bf16.
Host: shards weights per core, sums the 8 partial outputs in fp32.
"""
import sys
sys.path.insert(0, "/opt/trn_rl_repo")

import math
import numpy as np
import ml_dtypes

import concourse.bass as bass
import concourse.tile as tile
from concourse import bacc, mybir
from concourse.masks import make_identity

bf16 = ml_dtypes.bfloat16
F32 = mybir.dt.float32
BF = mybir.dt.bfloat16
AF = mybir.ActivationFunctionType
ALU = mybir.AluOpType

D = 2048          # model dim
NH = 16           # total heads
DH = 128          # head dim
NCORES = 8
HPC = NH // NCORES          # heads per core = 2
DLOC = HPC * DH             # local hdim = 256
EPS = 1e-6
TB = 512                    # t-block size
SQRT_DH = math.sqrt(DH)

# Newton rsqrt constants: rstd = 1/sqrt(a), a = ssq*SC + BI
RS_SC = SQRT_DH / DH
RS_BI = EPS * SQRT_DH
Y0 = 1.0 / math.sqrt(SQRT_DH * 0.25)   # seed at the expected a

_BUILD_CACHE = {}


def _build(T):
    """Build the per-core Bass program (same program on all cores)."""
    NTB = T // TB
    nc = bacc.Bacc("TRN2", target_bir_lowering=False)

    xt_in = nc.dram_tensor("xt", [D, T], BF, kind="ExternalInput")
    wq_in = nc.dram_tensor("wqkv", [D, 3 * DLOC], BF, kind="ExternalInput")
    wp_in = nc.dram_tensor("wproj", [DLOC, D], BF, kind="ExternalInput")
    ve_in = nc.dram_tensor("ve", [T, DLOC], BF, kind="ExternalInput")
    cos_in = nc.dram_tensor("cos", [T, 32], BF, kind="ExternalInput")
    sin_in = nc.dram_tensor("sin", [T, 32], BF, kind="ExternalInput")
    mask_in = nc.dram_tensor("mask", [128, 128], BF, kind="ExternalInput")
    out_d = nc.dram_tensor("out", [T, D], BF, kind="ExternalOutput")

    with tile.TileContext(nc) as tc:
        with (
            tc.tile_pool(name="const", bufs=1) as const,
            tc.tile_pool(name="res", bufs=1) as res,
            tc.tile_pool(name="xt", bufs=2) as xtp,
            tc.tile_pool(name="work", bufs=2) as work,
            tc.tile_pool(name="att", bufs=3) as att,
            tc.tile_pool(name="lac", bufs=2) as lac,
            tc.tile_pool(name="prj", bufs=2) as prj,
            tc.tile_pool(name="psA", bufs=2, space="PSUM") as psA,
            tc.tile_pool(name="psB", bufs=1, space="PSUM") as psB,
            tc.tile_pool(name="psC", bufs=1, space="PSUM") as psC,
            tc.tile_pool(name="psD", bufs=2, space="PSUM") as psD,
        ):
            # ---------------- constants ----------------
            wq_sb = const.tile([128, D // 128, 3 * DLOC], BF, tag="wq")
            nc.sync.dma_start(wq_sb[:], wq_in.rearrange("(c p) e -> p c e", p=128))
            wp_sb = const.tile([128, HPC, D], BF, tag="wp")
            nc.sync.dma_start(wp_sb[:], wp_in.rearrange("(h p) e -> p h e", p=128))
            cos_sb = const.tile([128, T // 128, 32], BF, tag="cos")
            nc.sync.dma_start(cos_sb[:], cos_in.rearrange("(c p) f -> p c f", p=128))
            sin_sb = const.tile([128, T // 128, 32], BF, tag="sin")
            nc.sync.dma_start(sin_sb[:], sin_in.rearrange("(c p) f -> p c f", p=128))
            mask_sb = const.tile([128, 128], BF, tag="mask")
            nc.sync.dma_start(mask_sb[:], mask_in[:])
            ident = const.tile([128, 128], BF, tag="ident")
            make_identity(nc, ident[:])
            ones = const.tile([128, 1], F32, tag="ones")
            nc.vector.memset(ones[:], 1.0)
            ones1 = const.tile([1, 128], BF, tag="ones1")
            nc.vector.memset(ones1[:], 1.0)

            # ---------------- resident per-block tensors ----------------
            qT = [res.tile([128, HPC, TB], BF, tag=f"qT{i}", name=f"qT{i}") for i in range(NTB)]
            kT = [res.tile([128, HPC, TB], BF, tag=f"kT{i}", name=f"kT{i}") for i in range(NTB)]
            vB = [res.tile([128, 4, DLOC], BF, tag=f"v{i}", name=f"v{i}") for i in range(NTB)]

            for ti in range(NTB):
                t0 = ti * TB
                # ============ QKV stage ============
                xt = xtp.tile([128, D // 128, TB], BF, tag="xt")
                nc.sync.dma_start(
                    xt[:], xt_in[:, t0:t0 + TB].rearrange("(c p) t -> p c t", p=128))

                for sub in range(4):
                    tg = ti * 4 + sub
                    qkv_ps = psA.tile([128, 1024], F32, tag="big")
                    ndc = D // 128
                    for dc in range(ndc):
                        lhsT = xt[:, dc, sub * 128:(sub + 1) * 128]
                        st, sp = dc == 0, dc == ndc - 1
                        nc.tensor.matmul(qkv_ps[:, 0:512], lhsT, wq_sb[:, dc, 0:512], start=st, stop=sp)
                        nc.tensor.matmul(qkv_ps[:, 512:768], lhsT, wq_sb[:, dc, 512:768], start=st, stop=sp)
                    # v (lambda0 pre-folded in weights; lambda1*ve DMA-accumulated below)
                    nc.any.tensor_copy(vB[ti][:, sub, :], qkv_ps[:, 512:768])
                    # rms statistics: sumsq per head for q and k (ACT Square, exp-set safe)
                    ssq = work.tile([128, 4], F32, tag="ssq")
                    sq_scr = work.tile([128, 512], BF, tag="sqscr")
                    for i in range(4):
                        nc.scalar.activation(
                            sq_scr[:, i * 128:(i + 1) * 128], qkv_ps[:, i * 128:(i + 1) * 128],
                            AF.Square, accum_out=ssq[:, i:i + 1])
                    # rstd = (sqrt(DH)*(ms+eps))^-1/2 via Newton on DVE (no table loads).
                    # hx = -0.5*a; iterate y <- y*(1.5 + hx*y^2)
                    hx = work.tile([128, 4], F32, tag="hx")
                    nc.vector.tensor_scalar(hx[:], ssq[:], -0.5 * RS_SC, -0.5 * RS_BI,
                                            op0=ALU.mult, op1=ALU.add)
                    y = work.tile([128, 4], F32, tag="yns")
                    nc.vector.memset(y[:], Y0)
                    u = work.tile([128, 4], F32, tag="uns")
                    v = work.tile([128, 4], F32, tag="vns")
                    for _ in range(4):
                        nc.vector.tensor_mul(u[:], y[:], y[:])
                        nc.vector.tensor_mul(v[:], u[:], hx[:])
                        nc.vector.tensor_scalar_add(v[:], v[:], 1.5)
                        nc.vector.tensor_mul(y[:], y[:], v[:])
                    # normalize + cast (combined q|k tile: j=0,1 -> q heads, 2,3 -> k heads)
                    qkn = work.tile([128, 4, DH], BF, tag="qkn")
                    for j in range(4):
                        nc.any.tensor_scalar_mul(qkn[:, j, :], qkv_ps[:, j * 128:(j + 1) * 128], y[:, j:j + 1])
                    # rope (first 32 freq pairs only; rest are identity), q and k fused
                    cosb = cos_sb[:, tg, :][:, None, :].broadcast_to([128, 4, 32])
                    sinb = sin_sb[:, tg, :][:, None, :].broadcast_to([128, 4, 32])
                    x1 = qkn[:, :, 0:32]
                    x2 = qkn[:, :, 64:96]
                    r1 = work.tile([128, 4, 32], BF, tag="r1")
                    r2 = work.tile([128, 4, 32], BF, tag="r2")
                    r3 = work.tile([128, 4, 32], BF, tag="r3")
                    r4 = work.tile([128, 4, 32], BF, tag="r4")
                    nc.vector.tensor_mul(r1[:], x1, cosb)
                    nc.vector.tensor_mul(r2[:], x2, sinb)
                    nc.vector.tensor_mul(r3[:], x1, sinb)
                    nc.vector.tensor_mul(r4[:], x2, cosb)
                    nc.vector.tensor_add(x1, r1[:], r2[:])
                    nc.vector.tensor_sub(x2, r4[:], r3[:])
                    # transpose q,k into resident [d, t] layout
                    for j in range(4):
                        tp = psC.tile([128, 512], BF, tag="tp")
                        nc.tensor.transpose(tp[:, 0:128], qkn[:, j, :], ident[:])
                        dstl = qT if j < 2 else kT
                        nc.any.tensor_copy(dstl[ti][:, j % 2, sub * 128:(sub + 1) * 128], tp[:, 0:128])
                # blend ve into v via accumulating DMA
                nc.gpsimd.dma_start(
                    vB[ti][:], ve_in[t0:t0 + TB, :].rearrange("(c p) d -> p c d", p=128),
                    accum_op=ALU.add)

                # ============ attention stage ============
                oB = prj.tile([128, HPC, TB], BF, tag="o")
                ns = (ti + 1) * 4
                for h in range(HPC):
                    l_acc = lac.tile([128, 1024], F32, tag="lacc")
                    nc.vector.memset(l_acc[:], 0.0)
                    o_ps = psB.tile([128, TB], F32, tag="o")
                    for sj2 in range(0, ns, 2):
                        js = [sj2 - ti * 4, sj2 + 1 - ti * 4]   # >=0 -> diagonal idx
                        offs = [max(0, 128 * j) for j in js]
                        sc_ps = psA.tile([128, 1024], F32, tag="big")
                        for k2 in range(2):
                            sj = sj2 + k2
                            blk, sb_ = sj // 4, sj % 4
                            nc.tensor.matmul(
                                sc_ps[:, k2 * 512 + offs[k2]:(k2 + 1) * 512],
                                kT[blk][:, h, sb_ * 128:(sb_ + 1) * 128],
                                qT[ti][:, h, offs[k2]:512], start=True, stop=True)
                        probs = att.tile([128, 1024], BF, tag="probs")
                        if offs == [0, 0]:
                            nc.scalar.activation(probs[:], sc_ps[:], AF.Exp)
                        else:
                            for k2 in range(2):
                                nc.scalar.activation(
                                    probs[:, k2 * 512 + offs[k2]:(k2 + 1) * 512],
                                    sc_ps[:, k2 * 512 + offs[k2]:(k2 + 1) * 512], AF.Exp)
                        for k2 in range(2):
                            j = js[k2]
                            if j >= 0:  # triangular mask on the diagonal 128-sub-block
                                sl = slice(k2 * 512 + 128 * j, k2 * 512 + 128 * (j + 1))
                                nc.vector.tensor_mul(probs[:, sl], probs[:, sl], mask_sb[:])
                        if offs == [0, 0]:
                            nc.vector.tensor_add(l_acc[:], l_acc[:], probs[:])
                        else:
                            for k2 in range(2):
                                sl = slice(k2 * 512 + offs[k2], (k2 + 1) * 512)
                                nc.vector.tensor_add(l_acc[:, sl], l_acc[:, sl], probs[:, sl])
                        for k2 in range(2):
                            sj = sj2 + k2
                            blk, sb_ = sj // 4, sj % 4
                            nc.tensor.matmul(
                                o_ps[:, offs[k2]:512], vB[blk][:, sb_, h * 128:(h + 1) * 128],
                                probs[:, k2 * 512 + offs[k2]:(k2 + 1) * 512],
                                start=(sj == 0), stop=(sj == ns - 1))
                    # l row-reduce on PE: lrow[0, q] = sum_kpos l_acc
                    lrow = psC.tile([128, 512], F32, tag="tp")
                    for c8 in range(8):
                        cc = c8 % 4
                        nc.tensor.matmul(lrow[0:1, cc * 128:(cc + 1) * 128], ones[:],
                                         l_acc[:, c8 * 128:(c8 + 1) * 128],
                                         start=(c8 == 0), stop=(c8 == 7))
                    linv_f = prj.tile([1, TB], F32, tag="linvf")
                    nc.vector.reciprocal_approx_fast(linv_f[:], lrow[0:1, :])
                    linv_row = prj.tile([1, TB], BF, tag="linvrow")
                    nc.vector.tensor_copy(linv_row[:], linv_f[:])
                    # broadcast 1/l across partitions via ones-outer-product
                    bc_ps = psC.tile([128, 512], F32, tag="tp")
                    nc.tensor.matmul(bc_ps[:], ones1[:], linv_row[:], start=True, stop=True)
                    bc_sb = prj.tile([128, TB], BF, tag="bcsb")
                    nc.vector.tensor_copy(bc_sb[:], bc_ps[:])
                    # o_scaled = o_ps * (1/l) broadcast  (psum x sbuf -> sbuf bf16)
                    nc.vector.tensor_mul(oB[:, h, :], o_ps[:], bc_sb[:])

                # ============ projection stage (both heads fused) ============
                for sub in range(4):
                    out_sb = prj.tile([128, D], BF, tag="outsb")
                    for dn in range(D // 512):
                        pr = psD.tile([128, 512], F32, tag="pr")
                        nc.tensor.matmul(pr[:], oB[:, 0, sub * 128:(sub + 1) * 128],
                                         wp_sb[:, 0, dn * 512:(dn + 1) * 512], start=True, stop=False)
                        nc.tensor.matmul(pr[:], oB[:, 1, sub * 128:(sub + 1) * 128],
                                         wp_sb[:, 1, dn * 512:(dn + 1) * 512], start=False, stop=True)
                        nc.any.tensor_copy(out_sb[:, dn * 512:(dn + 1) * 512], pr[:])
                    nc.sync.dma_start(out_d[t0 + sub * 128: t0 + (sub + 1) * 128, :], out_sb[:])
    return nc


def _host_prep(x, ve, lambdas, qkv_w, proj_w, T):
    """Build the 8 per-core input maps (sharding + constant tables)."""
    x = np.asarray(x, np.float32).reshape(T, D)
    xt = np.ascontiguousarray(x.T.astype(bf16))          # [D, T] bf16
    ve = np.asarray(ve, np.float32).reshape(T, NH * DH)
    lam = np.asarray(lambdas, np.float32)
    qkv_w = np.asarray(qkv_w, np.float32)
    proj_w = np.asarray(proj_w, np.float32)

    quarter = DH // 4
    ang = (1.0 / 1024.0) ** np.linspace(0.0, 1.0, quarter, dtype=np.float32)
    theta = np.arange(T, dtype=np.float32)[:, None] * ang[None, :]   # [T, 32]
    cos_t = np.cos(theta).astype(bf16)
    sin_t = np.sin(theta).astype(bf16)

    s_l = np.arange(128)[:, None]
    t_l = np.arange(128)[None, :]
    mask = (t_l >= s_l).astype(bf16)                     # [128,128] lower-tri in [s,t]

    in_maps = []
    for c in range(NCORES):
        sl = slice(c * DLOC, (c + 1) * DLOC)
        wqkv = np.concatenate(
            [qkv_w[0, sl].T, qkv_w[1, sl].T, lam[0] * qkv_w[2, sl].T], axis=1)  # [D, 768]
        in_maps.append({
            "xt": xt,
            "wqkv": np.ascontiguousarray(wqkv).astype(bf16),
            "wproj": np.ascontiguousarray(proj_w[:, sl].T).astype(bf16),
            "ve": np.ascontiguousarray(lam[1] * ve[:, sl]).astype(bf16),
            "cos": cos_t, "sin": sin_t, "mask": mask,
        })
    return in_maps


def kernel(x, ve, lambdas, qkv_w, proj_w):
    B, T, _ = x.shape
    in_maps = _host_prep(x, ve, lambdas, qkv_w, proj_w, T)
    if T not in _BUILD_CACHE:
        nc = _build(T)
        nc.compile()
        _BUILD_CACHE[T] = nc
    nc = _BUILD_CACHE[T]

    from concourse.bass_utils import run_bass_kernel_spmd
    res = run_bass_kernel_spmd(nc, in_maps, core_ids=list(range(NCORES)))
    out = np.zeros((T, D), np.float32)
    for c in range(NCORES):
        out += res.results[c]["out"].astype(np.float32)
    return out.reshape(B, T, D)


# revision 7
# speedup vs baseline: 1.5412x; 1.1064x over previous
"""Causal self-attention (RMS-normed QK + partial RoPE + lambda-blended V)
for Trainium2, tensor-parallel over heads across 8 NeuronCores.

Per core: 2 heads. Host pre-transposes x to xT (bf16). Device pipeline per
512-token block:
  xT strip DMA -> QKV matmuls (bf16, fp32 psum) -> immediate psum->SBUF
  evacuation (frees the shared psum slot fast) -> RMS via ACT square-accum
  + 9-op DVE Newton rsqrt (single activation-table set, no thrash) ->
  RoPE (DVE, fused q|k) -> PE-transpose into combined qkT resident tile ->
  flash-style causal attention with diagonal-block trimming (scores^T,
  ACT exp, triangular mask, two-stage l accumulation on DVE) -> l
  row-reduce + 1/l broadcast on PE -> pre-scaled o -> fused two-head
  output projection -> partial out [T, D] bf16.
Host: shards weights per core, sums the 8 partial outputs in fp32.
"""
import sys
sys.path.insert(0, "/opt/trn_rl_repo")

import math
import numpy as np
import ml_dtypes

import concourse.bass as bass
import concourse.tile as tile
from concourse import bacc, mybir
from concourse.masks import make_identity

bf16 = ml_dtypes.bfloat16
F32 = mybir.dt.float32
U32 = mybir.dt.uint32
BF = mybir.dt.bfloat16
AF = mybir.ActivationFunctionType
ALU = mybir.AluOpType

D = 2048          # model dim
NH = 16           # total heads
DH = 128          # head dim
NCORES = 8
HPC = NH // NCORES          # heads per core = 2
DLOC = HPC * DH             # local hdim = 256
EPS = 1e-6
TB = 512                    # t-block size
SQRT_DH = math.sqrt(DH)

# rstd = 1/sqrt(a), a = ssq*RS_SC + RS_BI
RS_SC = SQRT_DH / DH
RS_BI = EPS * SQRT_DH
RSQRT_MAGIC = 0x5F3759DF

_BUILD_CACHE = {}


def _build(T):
    """Build the per-core Bass program (same program on all cores)."""
    NTB = T // TB
    nc = bacc.Bacc("TRN2", target_bir_lowering=False)

    xt_in = nc.dram_tensor("xt", [D, T], BF, kind="ExternalInput")
    wq_in = nc.dram_tensor("wqkv", [D, 3 * DLOC], BF, kind="ExternalInput")
    wp_in = nc.dram_tensor("wproj", [DLOC, D], BF, kind="ExternalInput")
    ve_in = nc.dram_tensor("ve", [T, DLOC], BF, kind="ExternalInput")
    cos_in = nc.dram_tensor("cos", [T, 32], BF, kind="ExternalInput")
    sin_in = nc.dram_tensor("sin", [T, 32], BF, kind="ExternalInput")
    mask_in = nc.dram_tensor("mask", [128, 128], BF, kind="ExternalInput")
    out_d = nc.dram_tensor("out", [T, D], BF, kind="ExternalOutput")

    with tile.TileContext(nc) as tc:
        with (
            tc.tile_pool(name="const", bufs=1) as const,
            tc.tile_pool(name="res", bufs=1) as res,
            tc.tile_pool(name="xt", bufs=2) as xtp,
            tc.tile_pool(name="work", bufs=3) as work,
            tc.tile_pool(name="att", bufs=3) as att,
            tc.tile_pool(name="lac", bufs=2) as lac,
            tc.tile_pool(name="prj", bufs=2) as prj,
            tc.tile_pool(name="psA", bufs=2, space="PSUM") as psA,
            tc.tile_pool(name="psB", bufs=1, space="PSUM") as psB,
            tc.tile_pool(name="psC", bufs=1, space="PSUM") as psC,
            tc.tile_pool(name="psD", bufs=2, space="PSUM") as psD,
        ):
            # ---------------- constants ----------------
            wq_sb = const.tile([128, D // 128, 3 * DLOC], BF, tag="wq")
            nc.sync.dma_start(wq_sb[:], wq_in.rearrange("(c p) e -> p c e", p=128))
            wp_sb = const.tile([128, HPC, D], BF, tag="wp")
            nc.sync.dma_start(wp_sb[:], wp_in.rearrange("(h p) e -> p h e", p=128))
            cos_sb = const.tile([128, T // 128, 32], BF, tag="cos")
            nc.sync.dma_start(cos_sb[:], cos_in.rearrange("(c p) f -> p c f", p=128))
            sin_sb = const.tile([128, T // 128, 32], BF, tag="sin")
            nc.sync.dma_start(sin_sb[:], sin_in.rearrange("(c p) f -> p c f", p=128))
            mask_sb = const.tile([128, 128], BF, tag="mask")
            nc.sync.dma_start(mask_sb[:], mask_in[:])
            ident = const.tile([128, 128], BF, tag="ident")
            make_identity(nc, ident[:])
            ones = const.tile([128, 1], F32, tag="ones")
            nc.vector.memset(ones[:], 1.0)
            ones1 = const.tile([1, 128], BF, tag="ones1")
            nc.vector.memset(ones1[:], 1.0)

            # ---------------- resident per-block tensors ----------------
            # combined [q_h0, q_h1, k_h0, k_h1] transposed [d, t] store
            qkT = [res.tile([128, 4, TB], BF, tag=f"qkT{i}", name=f"qkT{i}") for i in range(NTB)]
            vB = [res.tile([128, 4, DLOC], BF, tag=f"v{i}", name=f"v{i}") for i in range(NTB)]

            for ti in range(NTB):
                t0 = ti * TB
                # ============ QKV stage ============
                xt = xtp.tile([128, D // 128, TB], BF, tag="xt")
                nc.sync.dma_start(
                    xt[:], xt_in[:, t0:t0 + TB].rearrange("(c p) t -> p c t", p=128))

                for sub in range(4):
                    tg = ti * 4 + sub
                    qkv_ps = psA.tile([128, 1024], F32, tag="big")
                    ndc = D // 128
                    for dc in range(ndc):
                        lhsT = xt[:, dc, sub * 128:(sub + 1) * 128]
                        st, sp = dc == 0, dc == ndc - 1
                        nc.tensor.matmul(qkv_ps[:, 0:512], lhsT, wq_sb[:, dc, 0:512], start=st, stop=sp)
                        nc.tensor.matmul(qkv_ps[:, 512:768], lhsT, wq_sb[:, dc, 512:768], start=st, stop=sp)
                    # fast psum evacuation: frees the shared psA slot quickly
                    qk_raw = work.tile([128, 512], BF, tag="qkraw")
                    nc.vector.tensor_copy(qk_raw[:], qkv_ps[:, 0:512])
                    nc.any.tensor_copy(vB[ti][:, sub, :], qkv_ps[:, 512:768])
                    # rms statistics: sumsq per head for q and k (ACT Square)
                    ssq = work.tile([128, 4], F32, tag="ssq")
                    sq_scr = work.tile([128, 512], BF, tag="sqscr")
                    for i in range(4):
                        nc.scalar.activation(
                            sq_scr[:, i * 128:(i + 1) * 128], qk_raw[:, i * 128:(i + 1) * 128],
                            AF.Square, accum_out=ssq[:, i:i + 1])
                    # rstd = 1/sqrt(ssq*RS_SC + RS_BI): linear seed + 3 Newton (DVE)
                    a = work.tile([128, 4], F32, tag="a_ns")
                    nc.vector.tensor_scalar(a[:], ssq[:], RS_SC, RS_BI, op0=ALU.mult, op1=ALU.add)
                    y = work.tile([128, 4], F32, tag="y_ns")
                    nc.vector.tensor_scalar(y[:], a[:], -0.095, 0.968, op0=ALU.mult, op1=ALU.add)
                    u = work.tile([128, 4], F32, tag="u_ns")
                    for _ in range(3):
                        nc.vector.tensor_mul(u[:], y[:], y[:])
                        nc.vector.scalar_tensor_tensor(u[:], u[:], -0.5, a[:], op0=ALU.mult, op1=ALU.mult)
                        nc.vector.scalar_tensor_tensor(y[:], u[:], 1.5, y[:], op0=ALU.add, op1=ALU.mult)
                    # normalize + cast (combined q|k tile: j=0,1 -> q heads, 2,3 -> k heads)
                    qkn = work.tile([128, 4, DH], BF, tag="qkn")
                    for j in range(4):
                        nc.any.tensor_scalar_mul(qkn[:, j, :], qk_raw[:, j * 128:(j + 1) * 128], y[:, j:j + 1])
                    # rope (first 32 freq pairs only; rest are identity), q and k fused
                    cosb = cos_sb[:, tg, :][:, None, :].broadcast_to([128, 4, 32])
                    sinb = sin_sb[:, tg, :][:, None, :].broadcast_to([128, 4, 32])
                    x1 = qkn[:, :, 0:32]
                    x2 = qkn[:, :, 64:96]
                    r1 = work.tile([128, 4, 32], BF, tag="r1")
                    r2 = work.tile([128, 4, 32], BF, tag="r2")
                    r3 = work.tile([128, 4, 32], BF, tag="r3")
                    r4 = work.tile([128, 4, 32], BF, tag="r4")
                    nc.vector.tensor_mul(r1[:], x1, cosb)
                    nc.vector.tensor_mul(r2[:], x2, sinb)
                    nc.vector.tensor_mul(r3[:], x1, sinb)
                    nc.vector.tensor_mul(r4[:], x2, cosb)
                    nc.vector.tensor_add(x1, r1[:], r2[:])
                    nc.vector.tensor_sub(x2, r4[:], r3[:])
                    # transpose q,k into the combined resident [d, j, t] layout
                    tp = psC.tile([128, 4, 128], BF, tag="tp")
                    for j in range(4):
                        nc.tensor.transpose(tp[:, j, :], qkn[:, j, :], ident[:])
                    nc.any.tensor_copy(qkT[ti][:, :, sub * 128:(sub + 1) * 128], tp[:])
                # blend ve into v via accumulating DMA
                nc.gpsimd.dma_start(
                    vB[ti][:], ve_in[t0:t0 + TB, :].rearrange("(c p) d -> p c d", p=128),
                    accum_op=ALU.add)

                # ============ attention stage ============
                oB = prj.tile([128, HPC, TB], BF, tag="o")
                ns = (ti + 1) * 4
                for h in range(HPC):
                    l_acc = lac.tile([128, TB], F32, tag="lacc")
                    nc.vector.memset(l_acc[:], 0.0)
                    o_ps = psB.tile([128, TB], F32, tag="o")
                    for sj2 in range(0, ns, 2):
                        js = [sj2 - ti * 4, sj2 + 1 - ti * 4]   # >=0 -> diagonal idx
                        offs = [max(0, 128 * j) for j in js]
                        sc_ps = psA.tile([128, 1024], F32, tag="big")
                        for k2 in range(2):
                            sj = sj2 + k2
                            blk, sb_ = sj // 4, sj % 4
                            nc.tensor.matmul(
                                sc_ps[:, k2 * 512 + offs[k2]:(k2 + 1) * 512],
                                qkT[blk][:, 2 + h, sb_ * 128:(sb_ + 1) * 128],
                                qkT[ti][:, h, offs[k2]:512], start=True, stop=True)
                        probs = att.tile([128, 1024], BF, tag="probs")
                        if offs == [0, 0]:
                            nc.scalar.activation(probs[:], sc_ps[:], AF.Exp)
                        else:
                            for k2 in range(2):
                                nc.scalar.activation(
                                    probs[:, k2 * 512 + offs[k2]:(k2 + 1) * 512],
                                    sc_ps[:, k2 * 512 + offs[k2]:(k2 + 1) * 512], AF.Exp)
                        for k2 in range(2):
                            j = js[k2]
                            if j >= 0:  # triangular mask on the diagonal 128-sub-block
                                sl = slice(k2 * 512 + 128 * j, k2 * 512 + 128 * (j + 1))
                                nc.vector.tensor_mul(probs[:, sl], probs[:, sl], mask_sb[:])
                        # two-stage l accumulation: bf16 pair-sum (2x mode), then fp32
                        if offs == [0, 0]:
                            lp = att.tile([128, TB], BF, tag="lpair")
                            nc.vector.tensor_add(lp[:], probs[:, 0:512], probs[:, 512:1024])
                            nc.vector.tensor_add(l_acc[:], l_acc[:], lp[:])
                        else:
                            for k2 in range(2):
                                sl = slice(k2 * 512 + offs[k2], (k2 + 1) * 512)
                                nc.vector.tensor_add(l_acc[:, offs[k2]:], l_acc[:, offs[k2]:], probs[:, sl])
                        for k2 in range(2):
                            sj = sj2 + k2
                            blk, sb_ = sj // 4, sj % 4
                            nc.tensor.matmul(
                                o_ps[:, offs[k2]:512], vB[blk][:, sb_, h * 128:(h + 1) * 128],
                                probs[:, k2 * 512 + offs[k2]:(k2 + 1) * 512],
                                start=(sj == 0), stop=(sj == ns - 1))
                    # l row-reduce on PE: lrow[0, q] = sum_kpos l_acc
                    lrow = psC.tile([128, 512], F32, tag="tp")
                    for c in range(4):
                        nc.tensor.matmul(lrow[0:1, c * 128:(c + 1) * 128], ones[:],
                                         l_acc[:, c * 128:(c + 1) * 128],
                                         start=(c == 0), stop=(c == 3))
                    linv_f = prj.tile([1, TB], F32, tag="linvf")
                    nc.vector.reciprocal_approx_fast(linv_f[:], lrow[0:1, :])
                    linv_row = prj.tile([1, TB], BF, tag="linvrow")
                    nc.vector.tensor_copy(linv_row[:], linv_f[:])
                    # broadcast 1/l across partitions via ones-outer-product
                    bc_ps = psC.tile([128, 512], F32, tag="tp")
                    nc.tensor.matmul(bc_ps[:], ones1[:], linv_row[:], start=True, stop=True)
                    bc_sb = prj.tile([128, TB], BF, tag="bcsb")
                    nc.vector.tensor_copy(bc_sb[:], bc_ps[:])
                    # o_scaled = o_ps * (1/l) broadcast  (psum x sbuf -> sbuf bf16)
                    nc.vector.tensor_mul(oB[:, h, :], o_ps[:], bc_sb[:])

                # ============ projection stage (both heads fused) ============
                for sub in range(4):
                    out_sb = prj.tile([128, D], BF, tag="outsb")
                    for dn in range(D // 512):
                        pr = psD.tile([128, 512], F32, tag="pr")
                        nc.tensor.matmul(pr[:], oB[:, 0, sub * 128:(sub + 1) * 128],
                                         wp_sb[:, 0, dn * 512:(dn + 1) * 512], start=True, stop=False)
                        nc.tensor.matmul(pr[:], oB[:, 1, sub * 128:(sub + 1) * 128],
                                         wp_sb[:, 1, dn * 512:(dn + 1) * 512], start=False, stop=True)
                        nc.any.tensor_copy(out_sb[:, dn * 512:(dn + 1) * 512], pr[:])
                    nc.sync.dma_start(out_d[t0 + sub * 128: t0 + (sub + 1) * 128, :], out_sb[:])
    return nc


def _host_prep(x, ve, lambdas, qkv_w, proj_w, T):
    """Build the 8 per-core input maps (sharding + constant tables)."""
    x = np.asarray(x, np.float32).reshape(T, D)
    xt = np.ascontiguousarray(x.T.astype(bf16))          # [D, T] bf16
    ve = np.asarray(ve, np.float32).reshape(T, NH * DH)
    lam = np.asarray(lambdas, np.float32)
    qkv_w = np.asarray(qkv_w, np.float32)
    proj_w = np.asarray(proj_w, np.float32)

    quarter = DH // 4
    ang = (1.0 / 1024.0) ** np.linspace(0.0, 1.0, quarter, dtype=np.float32)
    theta = np.arange(T, dtype=np.float32)[:, None] * ang[None, :]   # [T, 32]
    cos_t = np.cos(theta).astype(bf16)
    sin_t = np.sin(theta).astype(bf16)

    s_l = np.arange(128)[:, None]
    t_l = np.arange(128)[None, :]
    mask = (t_l >= s_l).astype(bf16)                     # [128,128] lower-tri in [s,t]

    in_maps = []
    for c in range(NCORES):
        sl = slice(c * DLOC, (c + 1) * DLOC)
        wqkv = np.concatenate(
            [qkv_w[0, sl].T, qkv_w[1, sl].T, lam[0] * qkv_w[2, sl].T], axis=1)  # [D, 768]
        in_maps.append({
            "xt": xt,
            "wqkv": np.ascontiguousarray(wqkv).astype(bf16),
            "wproj": np.ascontiguousarray(proj_w[:, sl].T).astype(bf16),
            "ve": np.ascontiguousarray(lam[1] * ve[:, sl]).astype(bf16),
            "cos": cos_t, "sin": sin_t, "mask": mask,
        })
    return in_maps


def kernel(x, ve, lambdas, qkv_w, proj_w):
    B, T, _ = x.shape
    in_maps = _host_prep(x, ve, lambdas, qkv_w, proj_w, T)
    if T not in _BUILD_CACHE:
        nc = _build(T)
        nc.compile()
        _BUILD_CACHE[T] = nc
    nc = _BUILD_CACHE[T]

    from concourse.bass_utils import run_bass_kernel_spmd
    res = run_bass_kernel_spmd(nc, in_maps, core_ids=list(range(NCORES)))
    out = np.zeros((T, D), np.float32)
    for c in range(NCORES):
        out += res.results[c]["out"].astype(np.float32)
    return out.reshape(B, T, D)


# revision 8
# speedup vs baseline: 1.6047x; 1.0412x over previous
"""Causal self-attention (RMS-normed QK + partial RoPE + lambda-blended V)
for Trainium2, tensor-parallel over heads across 8 NeuronCores.

Per core: 2 heads. Host pre-transposes x to xT (bf16). Device pipeline per
512-token block:
  xT strip DMA -> QKV matmuls (bf16, fp32 psum) -> immediate psum->SBUF
  evacuation (frees the shared psum slot fast) -> RMS via ACT square-accum
  + 9-op DVE Newton rsqrt (single activation-table set, no thrash) ->
  RoPE (DVE, fused q|k) -> PE-transpose into combined qkT resident tile ->
  flash-style causal attention with diagonal-block trimming (scores^T,
  ACT exp, triangular mask, two-stage l accumulation on DVE) -> l
  row-reduce + 1/l broadcast on PE -> pre-scaled o -> fused two-head
  output projection -> partial out [T, D] bf16.
Host: shards weights per core, sums the 8 partial outputs in fp32.
"""
import sys
sys.path.insert(0, "/opt/trn_rl_repo")

import math
import numpy as np
import ml_dtypes

import concourse.bass as bass
import concourse.tile as tile
from concourse import bacc, bass_isa, mybir
from concourse.masks import make_identity

bf16 = ml_dtypes.bfloat16
F32 = mybir.dt.float32
U32 = mybir.dt.uint32
BF = mybir.dt.bfloat16
AF = mybir.ActivationFunctionType
ALU = mybir.AluOpType

D = 2048          # model dim
NH = 16           # total heads
DH = 128          # head dim
NCORES = 8
HPC = NH // NCORES          # heads per core = 2
DLOC = HPC * DH             # local hdim = 256
EPS = 1e-6
TB = 512                    # t-block size
SQRT_DH = math.sqrt(DH)

# rstd = 1/sqrt(a), a = ssq*RS_SC + RS_BI
RS_SC = SQRT_DH / DH
RS_BI = EPS * SQRT_DH
RSQRT_MAGIC = 0x5F3759DF

_BUILD_CACHE = {}


def _build(T):
    """Build the per-core Bass program (same program on all cores)."""
    NTB = T // TB
    nc = bacc.Bacc("TRN2", target_bir_lowering=False)

    xt_in = nc.dram_tensor("xt", [D, T], BF, kind="ExternalInput")
    wq_in = nc.dram_tensor("wqkv", [D, 3 * DLOC], BF, kind="ExternalInput")
    wp_in = nc.dram_tensor("wproj", [DLOC, D], BF, kind="ExternalInput")
    ve_in = nc.dram_tensor("ve", [T, DLOC], BF, kind="ExternalInput")
    cos_in = nc.dram_tensor("cos", [T, 32], BF, kind="ExternalInput")
    sin_in = nc.dram_tensor("sin", [T, 32], BF, kind="ExternalInput")
    mask_in = nc.dram_tensor("mask", [128, 128], BF, kind="ExternalInput")
    out_d = nc.dram_tensor("out", [T, D], BF, kind="ExternalOutput")

    with tile.TileContext(nc) as tc:
        with (
            tc.tile_pool(name="const", bufs=1) as const,
            tc.tile_pool(name="res", bufs=1) as res,
            tc.tile_pool(name="xt", bufs=2) as xtp,
            tc.tile_pool(name="work", bufs=3) as work,
            tc.tile_pool(name="att", bufs=3) as att,
            tc.tile_pool(name="lac", bufs=2) as lac,
            tc.tile_pool(name="prj", bufs=2) as prj,
            tc.tile_pool(name="psA", bufs=2, space="PSUM") as psA,
            tc.tile_pool(name="psQ", bufs=1, space="PSUM") as psQ,
            tc.tile_pool(name="psB", bufs=1, space="PSUM") as psB,
            tc.tile_pool(name="psC", bufs=1, space="PSUM") as psC,
        ):
            # ---------------- constants ----------------
            wq_sb = const.tile([128, D // 128, 3 * DLOC], BF, tag="wq")
            nc.sync.dma_start(wq_sb[:], wq_in.rearrange("(c p) e -> p c e", p=128))
            wp_sb = const.tile([128, HPC, D], BF, tag="wp")
            nc.sync.dma_start(wp_sb[:], wp_in.rearrange("(h p) e -> p h e", p=128))
            cos_sb = const.tile([128, T // 128, 32], BF, tag="cos")
            nc.sync.dma_start(cos_sb[:], cos_in.rearrange("(c p) f -> p c f", p=128))
            sin_sb = const.tile([128, T // 128, 32], BF, tag="sin")
            nc.sync.dma_start(sin_sb[:], sin_in.rearrange("(c p) f -> p c f", p=128))
            mask_sb = const.tile([128, 128], BF, tag="mask")
            nc.sync.dma_start(mask_sb[:], mask_in[:])
            ident = const.tile([128, 128], BF, tag="ident")
            make_identity(nc, ident[:])

            # ---------------- resident per-block tensors ----------------
            # combined [q_h0, q_h1, k_h0, k_h1] transposed [d, t] store
            qkT = [res.tile([128, 4, TB], BF, tag=f"qkT{i}", name=f"qkT{i}") for i in range(NTB)]
            vB = [res.tile([128, 4, DLOC], BF, tag=f"v{i}", name=f"v{i}") for i in range(NTB)]

            for ti in range(NTB):
                t0 = ti * TB
                # ============ QKV stage ============
                xt = xtp.tile([128, D // 128, TB], BF, tag="xt")
                nc.sync.dma_start(
                    xt[:], xt_in[:, t0:t0 + TB].rearrange("(c p) t -> p c t", p=128))

                for sub in range(4):
                    tg = ti * 4 + sub
                    qkv_ps = psQ.tile([128, 768], F32, tag="qkv")
                    ndc = D // 128
                    for dc in range(ndc):
                        lhsT = xt[:, dc, sub * 128:(sub + 1) * 128]
                        st, sp = dc == 0, dc == ndc - 1
                        nc.tensor.matmul(qkv_ps[:, 0:512], lhsT, wq_sb[:, dc, 0:512], start=st, stop=sp)
                        nc.tensor.matmul(qkv_ps[:, 512:768], lhsT, wq_sb[:, dc, 512:768], start=st, stop=sp)
                    # fast psum evacuation: frees the shared psA slot quickly
                    qk_raw = work.tile([128, 512], BF, tag="qkraw")
                    nc.vector.tensor_copy(qk_raw[:], qkv_ps[:, 0:512])
                    nc.any.tensor_copy(vB[ti][:, sub, :], qkv_ps[:, 512:768])
                    # rms statistics: sumsq per head for q and k (ACT Square)
                    ssq = work.tile([128, 4], F32, tag="ssq")
                    sq_scr = work.tile([128, 512], BF, tag="sqscr")
                    for i in range(4):
                        nc.scalar.activation(
                            sq_scr[:, i * 128:(i + 1) * 128], qk_raw[:, i * 128:(i + 1) * 128],
                            AF.Square, accum_out=ssq[:, i:i + 1])
                    # rstd = 1/sqrt(ssq*RS_SC + RS_BI): linear seed + 3 Newton (DVE)
                    a = work.tile([128, 4], F32, tag="a_ns")
                    nc.vector.tensor_scalar(a[:], ssq[:], RS_SC, RS_BI, op0=ALU.mult, op1=ALU.add)
                    y = work.tile([128, 4], F32, tag="y_ns")
                    nc.vector.tensor_scalar(y[:], a[:], -0.095, 0.968, op0=ALU.mult, op1=ALU.add)
                    u = work.tile([128, 4], F32, tag="u_ns")
                    for _ in range(3):
                        nc.vector.tensor_mul(u[:], y[:], y[:])
                        nc.vector.scalar_tensor_tensor(u[:], u[:], -0.5, a[:], op0=ALU.mult, op1=ALU.mult)
                        nc.vector.scalar_tensor_tensor(y[:], u[:], 1.5, y[:], op0=ALU.add, op1=ALU.mult)
                    # normalize + cast (combined q|k tile: j=0,1 -> q heads, 2,3 -> k heads)
                    qkn = work.tile([128, 4, DH], BF, tag="qkn")
                    for j in range(4):
                        nc.any.tensor_scalar_mul(qkn[:, j, :], qk_raw[:, j * 128:(j + 1) * 128], y[:, j:j + 1])
                    # rope (first 32 freq pairs only; rest are identity), q and k fused
                    cosb = cos_sb[:, tg, :][:, None, :].broadcast_to([128, 4, 32])
                    sinb = sin_sb[:, tg, :][:, None, :].broadcast_to([128, 4, 32])
                    x1 = qkn[:, :, 0:32]
                    x2 = qkn[:, :, 64:96]
                    r1 = work.tile([128, 4, 32], BF, tag="r1")
                    r2 = work.tile([128, 4, 32], BF, tag="r2")
                    r3 = work.tile([128, 4, 32], BF, tag="r3")
                    r4 = work.tile([128, 4, 32], BF, tag="r4")
                    nc.vector.tensor_mul(r1[:], x1, cosb)
                    nc.vector.tensor_mul(r2[:], x2, sinb)
                    nc.vector.tensor_mul(r3[:], x1, sinb)
                    nc.vector.tensor_mul(r4[:], x2, cosb)
                    nc.vector.tensor_add(x1, r1[:], r2[:])
                    nc.vector.tensor_sub(x2, r4[:], r3[:])
                    # transpose q,k into the combined resident [d, j, t] layout
                    tp = psC.tile([128, 4, 128], BF, tag="tp")
                    for j in range(4):
                        nc.tensor.transpose(tp[:, j, :], qkn[:, j, :], ident[:])
                    nc.any.tensor_copy(qkT[ti][:, :, sub * 128:(sub + 1) * 128], tp[:])
                # blend ve into v via accumulating DMA
                nc.gpsimd.dma_start(
                    vB[ti][:], ve_in[t0:t0 + TB, :].rearrange("(c p) d -> p c d", p=128),
                    accum_op=ALU.add)

                # ============ attention stage ============
                oB = prj.tile([128, HPC, TB], BF, tag="o")
                ns = (ti + 1) * 4
                for h in range(HPC):
                    l_acc = lac.tile([128, TB], F32, tag="lacc")
                    nc.vector.memset(l_acc[:], 0.0)
                    o_ps = psB.tile([128, TB], F32, tag="o")
                    for sj2 in range(0, ns, 2):
                        js = [sj2 - ti * 4, sj2 + 1 - ti * 4]   # >=0 -> diagonal idx
                        offs = [max(0, 128 * j) for j in js]
                        sc_ps = psA.tile([128, 1024], F32, tag="big")
                        for k2 in range(2):
                            sj = sj2 + k2
                            blk, sb_ = sj // 4, sj % 4
                            nc.tensor.matmul(
                                sc_ps[:, k2 * 512 + offs[k2]:(k2 + 1) * 512],
                                qkT[blk][:, 2 + h, sb_ * 128:(sb_ + 1) * 128],
                                qkT[ti][:, h, offs[k2]:512], start=True, stop=True)
                        probs = att.tile([128, 1024], BF, tag="probs")
                        if offs == [0, 0]:
                            nc.scalar.activation(probs[:], sc_ps[:], AF.Exp)
                        else:
                            for k2 in range(2):
                                nc.scalar.activation(
                                    probs[:, k2 * 512 + offs[k2]:(k2 + 1) * 512],
                                    sc_ps[:, k2 * 512 + offs[k2]:(k2 + 1) * 512], AF.Exp)
                        for k2 in range(2):
                            j = js[k2]
                            if j >= 0:  # triangular mask on the diagonal 128-sub-block
                                sl = slice(k2 * 512 + 128 * j, k2 * 512 + 128 * (j + 1))
                                nc.vector.tensor_mul(probs[:, sl], probs[:, sl], mask_sb[:])
                        # two-stage l accumulation: bf16 pair-sum (2x mode), then fp32
                        if offs == [0, 0]:
                            lp = att.tile([128, TB], BF, tag="lpair")
                            nc.vector.tensor_add(lp[:], probs[:, 0:512], probs[:, 512:1024])
                            nc.vector.tensor_add(l_acc[:], l_acc[:], lp[:])
                        else:
                            for k2 in range(2):
                                sl = slice(k2 * 512 + offs[k2], (k2 + 1) * 512)
                                nc.vector.tensor_add(l_acc[:, offs[k2]:], l_acc[:, offs[k2]:], probs[:, sl])
                        for k2 in range(2):
                            sj = sj2 + k2
                            blk, sb_ = sj // 4, sj % 4
                            nc.tensor.matmul(
                                o_ps[:, offs[k2]:512], vB[blk][:, sb_, h * 128:(h + 1) * 128],
                                probs[:, k2 * 512 + offs[k2]:(k2 + 1) * 512],
                                start=(sj == 0), stop=(sj == ns - 1))
                    # l reduce+broadcast across partitions on GpSimd, then 1/l
                    lsum = lac.tile([128, TB], F32, tag="lsum")
                    nc.gpsimd.partition_all_reduce(lsum[:], l_acc[:], channels=128,
                                                   reduce_op=bass_isa.ReduceOp.add)
                    linv = lac.tile([128, TB], F32, tag="linv")
                    nc.vector.reciprocal_approx_fast(linv[:], lsum[:])
                    # o_scaled = o_ps * (1/l)  (psum x sbuf -> sbuf bf16)
                    nc.vector.tensor_mul(oB[:, h, :], o_ps[:], linv[:])

                # ============ projection stage (both heads fused) ============
                for sub in range(4):
                    out_sb = prj.tile([128, D], BF, tag="outsb")
                    for dn in range(D // 512):
                        pr = psB.tile([128, 512], F32, tag="o")
                        nc.tensor.matmul(pr[:], oB[:, 0, sub * 128:(sub + 1) * 128],
                                         wp_sb[:, 0, dn * 512:(dn + 1) * 512], start=True, stop=False)
                        nc.tensor.matmul(pr[:], oB[:, 1, sub * 128:(sub + 1) * 128],
                                         wp_sb[:, 1, dn * 512:(dn + 1) * 512], start=False, stop=True)
                        nc.any.tensor_copy(out_sb[:, dn * 512:(dn + 1) * 512], pr[:])
                    nc.sync.dma_start(out_d[t0 + sub * 128: t0 + (sub + 1) * 128, :], out_sb[:])
    return nc


def _host_prep(x, ve, lambdas, qkv_w, proj_w, T):
    """Build the 8 per-core input maps (sharding + constant tables)."""
    x = np.asarray(x, np.float32).reshape(T, D)
    xt = np.ascontiguousarray(x.T.astype(bf16))          # [D, T] bf16
    ve = np.asarray(ve, np.float32).reshape(T, NH * DH)
    lam = np.asarray(lambdas, np.float32)
    qkv_w = np.asarray(qkv_w, np.float32)
    proj_w = np.asarray(proj_w, np.float32)

    quarter = DH // 4
    ang = (1.0 / 1024.0) ** np.linspace(0.0, 1.0, quarter, dtype=np.float32)
    theta = np.arange(T, dtype=np.float32)[:, None] * ang[None, :]   # [T, 32]
    cos_t = np.cos(theta).astype(bf16)
    sin_t = np.sin(theta).astype(bf16)

    s_l = np.arange(128)[:, None]
    t_l = np.arange(128)[None, :]
    mask = (t_l >= s_l).astype(bf16)                     # [128,128] lower-tri in [s,t]

    in_maps = []
    for c in range(NCORES):
        sl = slice(c * DLOC, (c + 1) * DLOC)
        wqkv = np.concatenate(
            [qkv_w[0, sl].T, qkv_w[1, sl].T, lam[0] * qkv_w[2, sl].T], axis=1)  # [D, 768]
        in_maps.append({
            "xt": xt,
            "wqkv": np.ascontiguousarray(wqkv).astype(bf16),
            "wproj": np.ascontiguousarray(proj_w[:, sl].T).astype(bf16),
            "ve": np.ascontiguousarray(lam[1] * ve[:, sl]).astype(bf16),
            "cos": cos_t, "sin": sin_t, "mask": mask,
        })
    return in_maps


def kernel(x, ve, lambdas, qkv_w, proj_w):
    B, T, _ = x.shape
    in_maps = _host_prep(x, ve, lambdas, qkv_w, proj_w, T)
    if T not in _BUILD_CACHE:
        nc = _build(T)
        nc.compile()
        _BUILD_CACHE[T] = nc
    nc = _BUILD_CACHE[T]

    from concourse.bass_utils import run_bass_kernel_spmd
    res = run_bass_kernel_spmd(nc, in_maps, core_ids=list(range(NCORES)))
    out = np.zeros((T, D), np.float32)
    for c in range(NCORES):
        out += res.results[c]["out"].astype(np.float32)
    return out.reshape(B, T, D)


# revision 10
# speedup vs baseline: 1.6198x; 1.0094x over previous
"""Causal self-attention (RMS-normed QK + partial RoPE + lambda-blended V)
for Trainium2, tensor-parallel over heads across 8 NeuronCores.

Per core: 2 heads. Host pre-transposes x to xT (bf16). Device pipeline per
512-token block:
  xT strip DMA -> QKV matmuls (bf16, fp32 psum) -> immediate psum->SBUF
  evacuation (frees the shared psum slot fast) -> RMS via ACT square-accum
  + 9-op DVE Newton rsqrt (single activation-table set, no thrash) ->
  RoPE (DVE, fused q|k) -> PE-transpose into combined qkT resident tile ->
  flash-style causal attention with diagonal-block trimming (scores^T,
  ACT exp, triangular mask, two-stage l accumulation on DVE) -> l
  row-reduce + 1/l broadcast on PE -> pre-scaled o -> fused two-head
  output projection -> partial out [T, D] bf16.
Host: shards weights per core, sums the 8 partial outputs in fp32.
"""
import sys
sys.path.insert(0, "/opt/trn_rl_repo")

import math
import numpy as np
import ml_dtypes

import concourse.bass as bass
import concourse.tile as tile
from concourse import bacc, bass_isa, mybir
from concourse.masks import make_identity

bf16 = ml_dtypes.bfloat16
F32 = mybir.dt.float32
U32 = mybir.dt.uint32
BF = mybir.dt.bfloat16
AF = mybir.ActivationFunctionType
ALU = mybir.AluOpType

D = 2048          # model dim
NH = 16           # total heads
DH = 128          # head dim
NCORES = 8
HPC = NH // NCORES          # heads per core = 2
DLOC = HPC * DH             # local hdim = 256
EPS = 1e-6
TB = 512                    # t-block size
SQRT_DH = math.sqrt(DH)

# rstd = 1/sqrt(a), a = ssq*RS_SC + RS_BI
RS_SC = SQRT_DH / DH
RS_BI = EPS * SQRT_DH
RSQRT_MAGIC = 0x5F3759DF

_BUILD_CACHE = {}


def _build(T):
    """Build the per-core Bass program (same program on all cores)."""
    NTB = T // TB
    nc = bacc.Bacc("TRN2", target_bir_lowering=False)

    xt_in = nc.dram_tensor("xt", [D, T], BF, kind="ExternalInput")
    wq_in = nc.dram_tensor("wqkv", [D, 3 * DLOC], BF, kind="ExternalInput")
    wp_in = nc.dram_tensor("wproj", [DLOC, D], BF, kind="ExternalInput")
    ve_in = nc.dram_tensor("ve", [T, DLOC], BF, kind="ExternalInput")
    cos_in = nc.dram_tensor("cos", [T, 32], BF, kind="ExternalInput")
    sin_in = nc.dram_tensor("sin", [T, 32], BF, kind="ExternalInput")
    mask_in = nc.dram_tensor("mask", [128, 128], BF, kind="ExternalInput")
    out_d = nc.dram_tensor("out", [T, D], BF, kind="ExternalOutput")

    with tile.TileContext(nc) as tc:
        with (
            tc.tile_pool(name="const", bufs=1) as const,
            tc.tile_pool(name="res", bufs=1) as res,
            tc.tile_pool(name="xt", bufs=2) as xtp,
            tc.tile_pool(name="work", bufs=4) as work,
            tc.tile_pool(name="att", bufs=4) as att,
            tc.tile_pool(name="lac", bufs=2) as lac,
            tc.tile_pool(name="prj", bufs=2) as prj,
            tc.tile_pool(name="psA", bufs=2, space="PSUM") as psA,
            tc.tile_pool(name="psQ", bufs=1, space="PSUM") as psQ,
            tc.tile_pool(name="psB", bufs=1, space="PSUM") as psB,
            tc.tile_pool(name="psC", bufs=1, space="PSUM") as psC,
        ):
            # ---------------- constants ----------------
            wq_sb = const.tile([128, D // 128, 3 * DLOC], BF, tag="wq")
            nc.sync.dma_start(wq_sb[:], wq_in.rearrange("(c p) e -> p c e", p=128))
            wp_sb = const.tile([128, HPC, D], BF, tag="wp")
            nc.sync.dma_start(wp_sb[:], wp_in.rearrange("(h p) e -> p h e", p=128))
            cos_sb = const.tile([128, T // 128, 32], BF, tag="cos")
            nc.sync.dma_start(cos_sb[:], cos_in.rearrange("(c p) f -> p c f", p=128))
            sin_sb = const.tile([128, T // 128, 32], BF, tag="sin")
            nc.sync.dma_start(sin_sb[:], sin_in.rearrange("(c p) f -> p c f", p=128))
            mask_sb = const.tile([128, 128], BF, tag="mask")
            nc.sync.dma_start(mask_sb[:], mask_in[:])
            ident = const.tile([128, 128], BF, tag="ident")
            make_identity(nc, ident[:])

            # ---------------- resident per-block tensors ----------------
            # combined [q_h0, q_h1, k_h0, k_h1] transposed [d, t] store
            qkT = [res.tile([128, 4, TB], BF, tag=f"qkT{i}", name=f"qkT{i}") for i in range(NTB)]
            vB = [res.tile([128, 4, DLOC], BF, tag=f"v{i}", name=f"v{i}") for i in range(NTB)]

            for ti in range(NTB):
                t0 = ti * TB
                # ============ QKV stage ============
                xt = xtp.tile([128, D // 128, TB], BF, tag="xt")
                nc.sync.dma_start(
                    xt[:], xt_in[:, t0:t0 + TB].rearrange("(c p) t -> p c t", p=128))

                for sub in range(4):
                    tg = ti * 4 + sub
                    qkv_ps = psQ.tile([128, 768], F32, tag="qkv")
                    ndc = D // 128
                    for dc in range(ndc):
                        lhsT = xt[:, dc, sub * 128:(sub + 1) * 128]
                        st, sp = dc == 0, dc == ndc - 1
                        nc.tensor.matmul(qkv_ps[:, 0:512], lhsT, wq_sb[:, dc, 0:512], start=st, stop=sp)
                        nc.tensor.matmul(qkv_ps[:, 512:768], lhsT, wq_sb[:, dc, 512:768], start=st, stop=sp)
                    # fast psum evacuation: frees the shared psA slot quickly
                    qk_raw = work.tile([128, 512], BF, tag="qkraw")
                    nc.vector.tensor_copy(qk_raw[:], qkv_ps[:, 0:512])
                    nc.any.tensor_copy(vB[ti][:, sub, :], qkv_ps[:, 512:768])
                    # rms statistics: sumsq per head for q and k (ACT Square)
                    ssq = work.tile([128, 4], F32, tag="ssq")
                    sq_scr = work.tile([128, 512], BF, tag="sqscr")
                    for i in range(4):
                        nc.scalar.activation(
                            sq_scr[:, i * 128:(i + 1) * 128], qk_raw[:, i * 128:(i + 1) * 128],
                            AF.Square, accum_out=ssq[:, i:i + 1])
                    # rstd = 1/sqrt(ssq*RS_SC + RS_BI): linear seed + 3 Newton (DVE)
                    a = work.tile([128, 4], F32, tag="a_ns")
                    nc.vector.tensor_scalar(a[:], ssq[:], RS_SC, RS_BI, op0=ALU.mult, op1=ALU.add)
                    y = work.tile([128, 4], F32, tag="y_ns")
                    nc.vector.tensor_scalar(y[:], a[:], -0.095, 0.968, op0=ALU.mult, op1=ALU.add)
                    u = work.tile([128, 4], F32, tag="u_ns")
                    for _ in range(3):
                        nc.vector.tensor_mul(u[:], y[:], y[:])
                        nc.vector.scalar_tensor_tensor(u[:], u[:], -0.5, a[:], op0=ALU.mult, op1=ALU.mult)
                        nc.vector.scalar_tensor_tensor(y[:], u[:], 1.5, y[:], op0=ALU.add, op1=ALU.mult)
                    # normalize + cast (combined q|k tile: j=0,1 -> q heads, 2,3 -> k heads)
                    qkn = work.tile([128, 4, DH], BF, tag="qkn")
                    for j in range(4):
                        nc.any.tensor_scalar_mul(qkn[:, j, :], qk_raw[:, j * 128:(j + 1) * 128], y[:, j:j + 1])
                    # rope (first 32 freq pairs only; rest are identity), q and k fused
                    cosb = cos_sb[:, tg, :][:, None, :].broadcast_to([128, 4, 32])
                    sinb = sin_sb[:, tg, :][:, None, :].broadcast_to([128, 4, 32])
                    x1 = qkn[:, :, 0:32]
                    x2 = qkn[:, :, 64:96]
                    r1 = work.tile([128, 4, 32], BF, tag="r1")
                    r2 = work.tile([128, 4, 32], BF, tag="r2")
                    r3 = work.tile([128, 4, 32], BF, tag="r3")
                    r4 = work.tile([128, 4, 32], BF, tag="r4")
                    nc.vector.tensor_mul(r1[:], x1, cosb)
                    nc.vector.tensor_mul(r2[:], x2, sinb)
                    nc.vector.tensor_mul(r3[:], x1, sinb)
                    nc.vector.tensor_mul(r4[:], x2, cosb)
                    nc.vector.tensor_add(x1, r1[:], r2[:])
                    nc.vector.tensor_sub(x2, r4[:], r3[:])
                    # transpose q,k into the combined resident [d, j, t] layout
                    tp = psC.tile([128, 4, 128], BF, tag="tp")
                    for j in range(4):
                        nc.tensor.transpose(tp[:, j, :], qkn[:, j, :], ident[:])
                    nc.any.tensor_copy(qkT[ti][:, :, sub * 128:(sub + 1) * 128], tp[:])
                # blend ve into v via accumulating DMA
                nc.gpsimd.dma_start(
                    vB[ti][:], ve_in[t0:t0 + TB, :].rearrange("(c p) d -> p c d", p=128),
                    accum_op=ALU.add)

                # ============ attention stage ============
                oB = prj.tile([128, HPC, TB], BF, tag="o")
                ns = (ti + 1) * 4
                for h in range(HPC):
                    l_acc = lac.tile([128, TB], F32, tag="lacc")
                    nc.vector.memset(l_acc[:], 0.0)
                    o_ps = psB.tile([128, TB], F32, tag="o")
                    for sj2 in range(0, ns, 2):
                        js = [sj2 - ti * 4, sj2 + 1 - ti * 4]   # >=0 -> diagonal idx
                        offs = [max(0, 128 * j) for j in js]
                        sc_ps = psA.tile([128, 1024], F32, tag="big")
                        for k2 in range(2):
                            sj = sj2 + k2
                            blk, sb_ = sj // 4, sj % 4
                            nc.tensor.matmul(
                                sc_ps[:, k2 * 512 + offs[k2]:(k2 + 1) * 512],
                                qkT[blk][:, 2 + h, sb_ * 128:(sb_ + 1) * 128],
                                qkT[ti][:, h, offs[k2]:512], start=True, stop=True)
                        probs = att.tile([128, 1024], BF, tag="probs")
                        if offs == [0, 0]:
                            nc.scalar.activation(probs[:], sc_ps[:], AF.Exp)
                        else:
                            for k2 in range(2):
                                nc.scalar.activation(
                                    probs[:, k2 * 512 + offs[k2]:(k2 + 1) * 512],
                                    sc_ps[:, k2 * 512 + offs[k2]:(k2 + 1) * 512], AF.Exp)
                        for k2 in range(2):
                            j = js[k2]
                            if j >= 0:  # triangular mask on the diagonal 128-sub-block
                                sl = slice(k2 * 512 + 128 * j, k2 * 512 + 128 * (j + 1))
                                nc.vector.tensor_mul(probs[:, sl], probs[:, sl], mask_sb[:])
                        # two-stage l accumulation: bf16 pair-sum (2x mode), then fp32
                        if offs == [0, 0]:
                            lp = att.tile([128, TB], BF, tag="lpair")
                            nc.vector.tensor_add(lp[:], probs[:, 0:512], probs[:, 512:1024])
                            nc.vector.tensor_add(l_acc[:], l_acc[:], lp[:])
                        else:
                            for k2 in range(2):
                                sl = slice(k2 * 512 + offs[k2], (k2 + 1) * 512)
                                nc.vector.tensor_add(l_acc[:, offs[k2]:], l_acc[:, offs[k2]:], probs[:, sl])
                        for k2 in range(2):
                            sj = sj2 + k2
                            blk, sb_ = sj // 4, sj % 4
                            nc.tensor.matmul(
                                o_ps[:, offs[k2]:512], vB[blk][:, sb_, h * 128:(h + 1) * 128],
                                probs[:, k2 * 512 + offs[k2]:(k2 + 1) * 512],
                                start=(sj == 0), stop=(sj == ns - 1))
                    # l reduce+broadcast across partitions on GpSimd, then 1/l
                    lsum = lac.tile([128, TB], F32, tag="lsum")
                    nc.gpsimd.partition_all_reduce(lsum[:], l_acc[:], channels=128,
                                                   reduce_op=bass_isa.ReduceOp.add)
                    linv = lac.tile([128, TB], F32, tag="linv")
                    nc.vector.reciprocal_approx_fast(linv[:], lsum[:])
                    # o_scaled = o_ps * (1/l)  (psum x sbuf -> sbuf bf16)
                    nc.vector.tensor_mul(oB[:, h, :], o_ps[:], linv[:])

                # ============ projection stage (both heads fused) ============
                for sub in range(4):
                    out_sb = prj.tile([128, D], BF, tag="outsb")
                    for dn in range(D // 512):
                        pr = psB.tile([128, 512], F32, tag="o")
                        nc.tensor.matmul(pr[:], oB[:, 0, sub * 128:(sub + 1) * 128],
                                         wp_sb[:, 0, dn * 512:(dn + 1) * 512], start=True, stop=False)
                        nc.tensor.matmul(pr[:], oB[:, 1, sub * 128:(sub + 1) * 128],
                                         wp_sb[:, 1, dn * 512:(dn + 1) * 512], start=False, stop=True)
                        nc.any.tensor_copy(out_sb[:, dn * 512:(dn + 1) * 512], pr[:])
                    nc.sync.dma_start(out_d[t0 + sub * 128: t0 + (sub + 1) * 128, :], out_sb[:])
    return nc


def _host_prep(x, ve, lambdas, qkv_w, proj_w, T):
    """Build the 8 per-core input maps (sharding + constant tables)."""
    x = np.asarray(x, np.float32).reshape(T, D)
    xt = np.ascontiguousarray(x.T.astype(bf16))          # [D, T] bf16
    ve = np.asarray(ve, np.float32).reshape(T, NH * DH)
    lam = np.asarray(lambdas, np.float32)
    qkv_w = np.asarray(qkv_w, np.float32)
    proj_w = np.asarray(proj_w, np.float32)

    quarter = DH // 4
    ang = (1.0 / 1024.0) ** np.linspace(0.0, 1.0, quarter, dtype=np.float32)
    theta = np.arange(T, dtype=np.float32)[:, None] * ang[None, :]   # [T, 32]
    cos_t = np.cos(theta).astype(bf16)
    sin_t = np.sin(theta).astype(bf16)

    s_l = np.arange(128)[:, None]
    t_l = np.arange(128)[None, :]
    mask = (t_l >= s_l).astype(bf16)                     # [128,128] lower-tri in [s,t]

    in_maps = []
    for c in range(NCORES):
        sl = slice(c * DLOC, (c + 1) * DLOC)
        wqkv = np.concatenate(
            [qkv_w[0, sl].T, qkv_w[1, sl].T, lam[0] * qkv_w[2, sl].T], axis=1)  # [D, 768]
        in_maps.append({
            "xt": xt,
            "wqkv": np.ascontiguousarray(wqkv).astype(bf16),
            "wproj": np.ascontiguousarray(proj_w[:, sl].T).astype(bf16),
            "ve": np.ascontiguousarray(lam[1] * ve[:, sl]).astype(bf16),
            "cos": cos_t, "sin": sin_t, "mask": mask,
        })
    return in_maps


def kernel(x, ve, lambdas, qkv_w, proj_w):
    B, T, _ = x.shape
    in_maps = _host_prep(x, ve, lambdas, qkv_w, proj_w, T)
    if T not in _BUILD_CACHE:
        nc = _build(T)
        nc.compile()
        _BUILD_CACHE[T] = nc
    nc = _BUILD_CACHE[T]

    from concourse.bass_utils import run_bass_kernel_spmd
    res = run_bass_kernel_spmd(nc, in_maps, core_ids=list(range(NCORES)))
    out = np.zeros((T, D), np.float32)
    for c in range(NCORES):
        out += res.results[c]["out"].astype(np.float32)
    return out.reshape(B, T, D)
